# revision 35
# baseline (speedup 1.0000x reference)
"""AgreementRouter (dynamic capsule routing, 3 iterations) on 8 trn2 cores.

Math (reference simplified): priors are constant along F, so routing logits are
L[b,n,c] (init 0). Per iteration i:
    a = softmax_c(L)                      # uniform 1/C at i=0
    o[b,c,f] = sum_n a[b,n,c] x[b,n,c,f] + bias[c,f]
    if i < 2: L[b,n,c] += sum_f x[b,n,c,f] o[b,c,f]
Return o from the last iteration: [B, C, F].

Sharding: batch dim B=64 across 8 cores (8 per core, batch-local routing).

v3 design (vs v1 baseline at ~183us traced / this at ~164us):
 - column order is (c outer, f inner) == natural host layout; no host transpose.
 - NO fp32 matmuls (they cost 4 cycles/col on the PE): iter-0 bias rides a 1x1
   fp16 ones-matmul into PSUM; the Gram diag-extract (mask product) runs on
   GpSimd in fp16; o rows are partition-broadcast via DRAM-bounce DMAs.
 - l_update: fp16 DVE multiply (2x mode) + contiguous pairwise tree over the
   innermost f axis, tree/softmax ops batched over groups of G=2 batches to
   amortize DVE instruction overheads (~300ns each). Second iteration's
   accumulate into L runs on GpSimd.
 - softmax: DVE reduce_max(negate) -> DVE add -> ACT Exp (fp16) -> fp16
   reduce_sum -> reciprocal -> fp16 normalize, all batched per group.
 - G1_FP8 env flag (default off) keeps an fp8e4 DoubleRow path for the iter-1
   Gram: measured neutral-to-worse on HW (extra DMA + casts) and rel err
   ~1.5e-2 vs gate 2e-2, so it stays disabled.
"""

import sys

sys.path.insert(0, "/opt/trn_rl_repo")

import numpy as np
import ml_dtypes

import concourse.bass as bass
import concourse.bacc as bacc
import concourse.tile as tile
from concourse import mybir

B, N, C, F = 64, 1152, 32, 16
CF = C * F  # 512
P = 128
NT = N // P  # 9 n-tiles per batch row
NCORES = 8
BLOC = B // NCORES  # 8 batches per core

F32 = mybir.dt.float32
F16 = mybir.dt.float16
F8 = mybir.dt.float8e4
AX_X = mybir.AxisListType.X
MUL = mybir.AluOpType.mult
ADD = mybir.AluOpType.add
DR = mybir.MatmulPerfMode.DoubleRow
import os
G1_FP8 = os.environ.get("G1_FP8", "0") == "1"  # iter-1 Gram in fp8 DoubleRow (logit path only)


def build_bass(compile=True):
    nc = bacc.Bacc("TRN2")

    x_dram = nc.dram_tensor("x", [BLOC, N, CF], F16, kind="ExternalInput")
    mask_dram = nc.dram_tensor("mask", [C, CF], F16, kind="ExternalInput")
    bias_dram = nc.dram_tensor("bias_row", [1, CF], F32, kind="ExternalInput")
    out_dram = nc.dram_tensor("out", [BLOC, CF], F32, kind="ExternalOutput")
    if G1_FP8:
        x8_dram = nc.dram_tensor("x8", [BLOC, N, CF], F8, kind="ExternalInput")

    with tile.TileContext(nc) as tc:
        with (
            tc.tile_pool(name="xpool", bufs=1) as xpool,
            tc.tile_pool(name="lpool", bufs=1) as lpool,
            tc.tile_pool(name="apool", bufs=1) as apool,
            tc.tile_pool(name="singles", bufs=1) as singles,
            tc.tile_pool(name="work", bufs=1) as work,
            tc.tile_pool(name="obuf", bufs=3) as obuf,
            tc.tile_pool(name="small", bufs=8) as small,
            tc.tile_pool(name="ps_full", bufs=3, space="PSUM") as ps_full,
            tc.tile_pool(name="ps_b", bufs=2, space="PSUM") as ps_bcast,
            tc.tile_pool(name="ps_s", bufs=2, space="PSUM") as ps_small,
            tc.tile_pool(name="dramsc", bufs=2, space="DRAM") as dramsc,
        ):
            # ---- constants ----
            w0c = singles.tile([P, 1], F16, tag="w0c", name="w0c")  # all 1/C
            nc.vector.memset(w0c, 1.0 / C)
            ones33 = singles.tile([C + 1, P], F16, tag="ones33", name="ones33")
            nc.vector.memset(ones33, 1.0)
            mask_sb = singles.tile([C, CF], F16, tag="mask", name="mask")
            nc.sync.dma_start(out=mask_sb, in_=mask_dram[:])
            bias_sb = singles.tile([1, CF], F32, tag="bias", name="bias")
            nc.sync.dma_start(out=bias_sb, in_=bias_dram[:])
            bias16 = singles.tile([1, CF], F16, tag="bias16", name="bias16")
            nc.vector.tensor_copy(out=bias16, in_=bias_sb)
            one11 = singles.tile([1, 1], F16, tag="one11", name="one11")
            nc.vector.memset(one11, 1.0)
            # [33, CF] fp16 mask-product tiles; row 32 holds the bias so the
            # ones-matmul does diag-sum + bias add + broadcast in one shot.
            msk33 = []
            for i in range(3):
                t = singles.tile(
                    [C + 1, CF], F16, tag=f"msk33_{i}", name=f"msk33_{i}"
                )
                nc.vector.tensor_copy(out=t[C : C + 1, :], in_=bias16)
                msk33.append(t)

            # ---- persistent per-b tiles ----
            xt = [
                xpool.tile([P, NT, CF], F16, tag=f"x_{b}", name=f"x_{b}")
                for b in range(BLOC)
            ]
            G = 2
            NG = BLOC // G  # 2 groups of 4 b's; DVE tree/softmax ops batch per group
            Lg = [
                lpool.tile([P, G, NT, C], F32, tag=f"L_{g}", name=f"L_{g}")
                for g in range(NG)
            ]
            ag = [
                apool.tile([P, G, NT, C], F16, tag=f"a_{g}", name=f"a_{g}")
                for g in range(NG)
            ]
            at = [ag[b // G][:, b % G] for b in range(BLOC)]
            if G1_FP8:
                x8t = [
                    xpool.tile([P, NT, CF], F8, tag=f"x8_{b}", name=f"x8_{b}")
                    for b in range(BLOC)
                ]
                at8 = [
                    apool.tile([P, NT, C], F8, tag=f"a8_{b}", name=f"a8_{b}")
                    for b in range(BLOC)
                ]

            # ---- phase A: load x; iter-0 n-sums -> [1,CF] PSUM; +bias on DVE;
            # fp16 row DMA-replicated across partitions via a DRAM bounce.
            o_sb = [None] * BLOC
            for b in range(BLOC):
                src = x_dram[b].rearrange("(t p) cf -> p t cf", p=P)
                nc.sync.dma_start(out=xt[b], in_=src)
                if G1_FP8:
                    src8 = x8_dram[b].rearrange("(t p) cf -> p t cf", p=P)
                    nc.sync.dma_start(out=x8t[b], in_=src8)
                ps = ps_small.tile([1, CF], F32, tag="row_ps", name="o0ps")
                nc.tensor.matmul(
                    ps, lhsT=one11, rhs=bias16, start=True, stop=False
                )
                for t in range(NT):
                    nc.tensor.matmul(
                        ps,
                        lhsT=w0c,
                        rhs=xt[b][:, t, :],
                        start=False,
                        stop=(t == NT - 1),
                    )
                o0row = obuf.tile([1, CF], F16, tag="o0row", name="o0row")
                nc.scalar.copy(out=o0row, in_=ps)
                o0d = dramsc.tile([1, CF], F16, name="o0d")
                nc.sync.dma_start(out=o0d, in_=o0row)
                o16b = obuf.tile([P, CF], F16, tag="o16", name="o16")
                nc.sync.dma_start(
                    out=o16b,
                    in_=bass.AP(
                        tensor=o0d.tensor,
                        offset=o0d.offset,
                        ap=[[0, P]] + list(o0d.ap[1:]),
                    ),
                )
                o_sb[b] = o16b

            def l_update_group(g, o16s, first):
                """L[g] (+)= sum_f x*o for the 4 b's of group g.  Multiply is
                per-b (o broadcast needs rank 3); the pairwise tree over the
                innermost f axis is batched across the group (fp16 2x)."""
                prod = work.tile([P, G, NT, C, F], F16, tag="prod", name="prod")
                for j in range(G):
                    b = g * G + j
                    x4 = xt[b].rearrange("p t (c f) -> p t c f", f=F)
                    o4 = o16s[j].rearrange("p (c f) -> p c f", f=F)
                    nc.vector.tensor_tensor(
                        prod[:, j], x4, o4[:, None, :, :].to_broadcast([P, NT, C, F]),
                        MUL,
                    )
                pf = prod.rearrange("p g t c f -> p (g t c) f")
                h1 = work.tile([P, G * NT * C, F // 2], F16, tag="h1", name="h1")
                nc.vector.tensor_tensor(h1, pf[:, :, 0:8], pf[:, :, 8:16], ADD)
                h2 = work.tile([P, G * NT * C, F // 4], F16, tag="h2", name="h2")
                nc.vector.tensor_tensor(h2, h1[:, :, 0:4], h1[:, :, 4:8], ADD)
                h3 = work.tile([P, G * NT * C, 2], F16, tag="h3", name="h3")
                nc.vector.tensor_tensor(h3, h2[:, :, 0:2], h2[:, :, 2:4], ADD)
                Lf = Lg[g].rearrange("p g t c -> p (g t c)")
                if first:
                    nc.vector.tensor_tensor(Lf, h3[:, :, 0], h3[:, :, 1], ADD)
                else:
                    gg = work.tile([P, G * NT * C], F32, tag="g", name="g")
                    nc.vector.tensor_tensor(gg, h3[:, :, 0], h3[:, :, 1], ADD)
                    nc.gpsimd.tensor_tensor(Lf, Lf, gg, ADD)

            def softmax_group(g, a_out_g):
                GT = G * NT
                Lv = Lg[g].rearrange("p g t c -> p (g t) c")
                av = a_out_g.rearrange("p g t c -> p (g t) c")
                negmax = small.tile([P, GT], F32, tag="negmax", name="negmax")
                nc.vector.reduce_max(negmax, Lv, axis=AX_X, negate=True)
                el = work.tile([P, GT, C], F16, tag="el", name="el")
                nc.vector.tensor_tensor(
                    el, Lv, negmax[:, :, None].to_broadcast([P, GT, C]), ADD
                )
                e = work.tile([P, GT, C], F16, tag="e", name="e")
                nc.scalar.activation(
                    out=e, in_=el, func=mybir.ActivationFunctionType.Exp
                )
                z = small.tile([P, GT], F16, tag="z", name="z")
                with nc.allow_low_precision(reason="z=sum of 32 fp16 exps <= 32"):
                    nc.vector.reduce_sum(z, e, axis=AX_X)
                rz = small.tile([P, GT], F16, tag="rz", name="rz")
                with nc.allow_low_precision(reason="softmax scale, 1e-3 rel ok"):
                    nc.vector.reciprocal(rz, z)
                nc.vector.tensor_tensor(
                    av, e, rz[:, :, None].to_broadcast([P, GT, C]), MUL
                )

            def gram_extract(b, final):
                """a-weighted sum over n on the PE; diag+bias via fp16
                mask-product (GpSimd) + ones-matmul -> [1,CF] row; non-final
                broadcasts the row across partitions via a DRAM bounce."""
                full = ps_full.tile([C, CF], F32, tag="full", name="full")
                if (not final) and G1_FP8:
                    # fp8 DoubleRow: two n-tiles per instruction
                    for t in range(0, NT - 1, 2):
                        nc.tensor.matmul(
                            full,
                            lhsT=at8[b][:, t : t + 2, :],
                            rhs=x8t[b][:, t : t + 2, :],
                            start=(t == 0),
                            stop=False,
                            perf_mode=DR,
                        )
                    nc.tensor.matmul(
                        full,
                        lhsT=at8[b][:, NT - 1, :],
                        rhs=x8t[b][:, NT - 1, :],
                        start=False,
                        stop=True,
                    )
                else:
                    for t in range(NT):
                        nc.tensor.matmul(
                            full,
                            lhsT=at[b][:, t, :],
                            rhs=xt[b][:, t, :],
                            start=(t == 0),
                            stop=(t == NT - 1),
                        )
                msk = msk33[b % 3]
                if final:
                    # tail: DVE is idle once phase D ends; one PSUM-read TT
                    # replaces the ACT copy + GpSimd mask (saves ~2.6us of
                    # chain latency per b right where it sets the span)
                    nc.vector.tensor_tensor(msk[0:C, :], full, mask_sb, MUL)
                else:
                    fullsb = obuf.tile([C, CF], F16, tag="fullsb", name="fullsb")
                    nc.scalar.copy(out=fullsb, in_=full)
                    nc.gpsimd.tensor_tensor(msk[0:C, :], fullsb, mask_sb, MUL)
                ops = ps_small.tile([1, CF], F32, tag="row_ps", name="ops")
                nc.tensor.matmul(
                    ops, lhsT=ones33[:, 0:1], rhs=msk, start=True, stop=True
                )
                if final:
                    orow = obuf.tile([1, CF], F32, tag="orow", name="orow")
                    nc.scalar.copy(out=orow, in_=ops)
                    nc.sync.dma_start(out=out_dram[b : b + 1, :], in_=orow)
                    return None
                o1row = obuf.tile([1, CF], F16, tag="o0row", name="o1row")
                nc.scalar.copy(out=o1row, in_=ops)
                o1d = dramsc.tile([1, CF], F16, name="o1d")
                nc.sync.dma_start(out=o1d, in_=o1row)
                o16 = obuf.tile([P, CF], F16, tag="o16c", name="o16c")
                nc.sync.dma_start(
                    out=o16,
                    in_=bass.AP(
                        tensor=o1d.tensor,
                        offset=o1d.offset,
                        ap=[[0, P]] + list(o1d.ap[1:]),
                    ),
                )
                return o16

            # ---- phase B: iter-0 l_update + softmax (per group) ----
            for g in range(NG):
                l_update_group(g, o_sb[g * G : (g + 1) * G], first=True)
                softmax_group(g, ag[g])
                if G1_FP8:
                    for j in range(G):
                        b = g * G + j
                        nc.scalar.copy(out=at8[b], in_=at[b])
            # ---- phase C: iter-1 outputs ----
            o_c = [None] * BLOC
            for b in range(BLOC):
                o_c[b] = gram_extract(b, final=False)
            # ---- phase D: iter-1 l_update + softmax ----
            for g in range(NG):
                l_update_group(g, o_c[g * G : (g + 1) * G], first=False)
                softmax_group(g, ag[g])
            # ---- phase E: iter-2 outputs + store ----
            for b in range(BLOC):
                gram_extract(b, final=True)

    if compile:
        nc.compile()
    return nc


_NC_CACHE = None


def _get_nc():
    global _NC_CACHE
    if _NC_CACHE is None:
        _NC_CACHE = build_bass()
    return _NC_CACHE


def _make_mask():
    # column order (c, f): column index = c*F + f
    m = np.zeros((C, CF), dtype=np.float16)
    for c in range(C):
        m[c, c * F : (c + 1) * F] = 1.0
    return m


def _install_ntff_hook():
    """Provide antenv.axon_hooks (absent in this image) so bass_utils'
    trace=True path can capture NTFF profiles via libaxon's C ABI."""
    import contextlib
    import ctypes
    import types

    if "antenv.axon_hooks" in sys.modules:
        return
    try:
        from antenv.axon_hooks import get_axon_ntff_profile_hook  # noqa: F401

        return
    except ImportError:
        pass

    so_path = "/opt/axon/libaxon_pjrt.so"
    try:
        lib = ctypes.CDLL(so_path)
    except OSError:
        return
    if not hasattr(lib, "axon_start_nrt_profile"):
        return
    lib.axon_start_nrt_profile.argtypes = [
        ctypes.POINTER(ctypes.c_int64),
        ctypes.c_size_t,
    ]
    lib.axon_start_nrt_profile.restype = ctypes.c_int64
    lib.axon_stop_nrt_profile.argtypes = [ctypes.c_char_p]
    lib.axon_stop_nrt_profile.restype = ctypes.c_int64

    @contextlib.contextmanager
    def _hook(output_dir, device_ids):
        import jax

        jax.devices()
        if device_ids:
            ids = (ctypes.c_int64 * len(device_ids))(*device_ids)
            rc = lib.axon_start_nrt_profile(ids, len(device_ids))
        else:
            rc = lib.axon_start_nrt_profile(None, 0)
        if rc != 0:
            raise RuntimeError(f"axon_start_nrt_profile rc={rc}")
        try:
            yield
        finally:
            n = lib.axon_stop_nrt_profile(str(output_dir).encode())
            print(f"profile: {n} file(s) written to {output_dir}")

    mod = types.ModuleType("antenv.axon_hooks")
    mod.get_axon_ntff_profile_hook = lambda: _hook
    mod.set_axon_ntff_profile_hook = lambda h: None
    sys.modules["antenv.axon_hooks"] = mod


def _run(inputs, bias, trace=False):
    import concourse.bass_utils as bu
    from concourse.bass_utils import run_bass_kernel_spmd

    if trace:
        _install_ntff_hook()
        bu.upload_artifacts = lambda tmpdir: tmpdir  # no Fish bucket here

    # device columns ordered (c, f) — the natural host layout
    xf = np.asarray(inputs, dtype=np.float32).reshape(B, N, CF)
    x16 = xf.astype(np.float16)
    bias_row = np.asarray(bias, dtype=np.float32).reshape(1, CF)
    mask = _make_mask()
    in_maps = [
        {
            "x": x16[i * BLOC : (i + 1) * BLOC],
            "mask": mask,
            "bias_row": bias_row,
        }
        for i in range(NCORES)
    ]
    if G1_FP8:
        x8 = xf.astype(ml_dtypes.float8_e4m3)
        for i in range(NCORES):
            in_maps[i]["x8"] = x8[i * BLOC : (i + 1) * BLOC]
    nc = _get_nc()
    res = run_bass_kernel_spmd(nc, in_maps, core_ids=list(range(NCORES)), trace=trace)
    out = np.concatenate(
        [r["out"].reshape(BLOC, C, F) for r in res.results], axis=0
    )
    return out.astype(np.float32), res


def kernel(**inputs) -> np.ndarray:
    out, _ = _run(inputs["inputs"], inputs["bias"], trace=False)
    return out


def kernel_traced(**inputs):
    out, res = _run(inputs["inputs"], inputs["bias"], trace=True)
    return out, res
